# revision 20
# baseline (speedup 1.0000x reference)
"""Banded-causal complex attention on 8 Trainium2 NeuronCores.

Strategy: data-parallel over batch (B=8 -> 1 batch per core). Per core:
  - host feeds x[b].T pre-interleaved as xtr[n, p, c, j] so one 1MB DMA piece
    delivers ALL contraction chunks for one 512-column range with 8KB
    descriptors (max DMA throughput), letting compute pipeline behind DMA.
  - Q is packed [Wqr|Wqi]*scale^2*temp, K is packed [Wkr|-Wki]: the complex
    score real part (qr.kr - qi.ki)*scale*temp becomes ONE K=128 matmul.
  - matmuls run in float32r (single-pass fp32 PE mode; needs even free dims
    and moving-dim >= 256 for the fast path).
  - scores are computed transposed: sT_kb[key c, query r] covers the two
    query blocks (kb, kb+1) that attend key block kb, one N=256 matmul each.
  - band+causal masking is two triangular affine_selects on GpSimd over the
    halves of exp(sT).
  - softmax skips the max-subtraction (scores are O(5); masked entries are
    exactly zero) and row-sums ride along as a ones column appended to V.
  - emission is software-pipelined: per 512-column piece, project q/k/v,
    transpose v, score new key blocks, attend two blocks behind.
"""

import numpy as np

B, S, D, KD = 8, 2048, 512, 64
P = 128              # partition size / query block
NB = S // P          # 16 query/key blocks
DCH = D // P         # 4 contraction chunks
NCH = 4              # column pieces
NSL = S // NCH       # 512 columns per piece
NCORES = 8

_CACHE = {}
TRACE_KWARGS = {}    # test harness may set e.g. {"trace": True, "tmpdir": ...}


def _build_nc():
    import concourse.bacc as bacc
    import concourse.tile as tile
    import concourse.mybir as mybir
    from concourse.bass import ts

    f32 = mybir.dt.float32
    f32r = mybir.dt.float32r
    nc = bacc.Bacc(None)

    xtr = nc.declare_dram_parameter("xtr", [NCH, P, DCH, NSL], f32r, isOutput=False)
    wq = nc.declare_dram_parameter("wq", [P, DCH, P], f32r, isOutput=False)
    wk = nc.declare_dram_parameter("wk", [P, DCH, P], f32r, isOutput=False)
    wv = nc.declare_dram_parameter("wv", [P, DCH, KD], f32r, isOutput=False)
    pq = nc.declare_dram_parameter("pq", [P, S], f32, isOutput=False)
    pk = nc.declare_dram_parameter("pk", [P, S], f32, isOutput=False)
    bv = nc.declare_dram_parameter("bv", [KD, 1], f32, isOutput=False)
    out = nc.declare_dram_parameter("out", [S, KD], f32, isOutput=True)

    ident = nc.inline_tensor(np.eye(KD, dtype=np.float32), name="ident64")
    cc, rr = np.meshgrid(np.arange(P), np.arange(P), indexing="ij")
    msk_np = np.stack([(cc <= rr), (cc >= rr)], axis=1).astype(np.float32)
    msk = nc.inline_tensor(msk_np, name="trimask")  # [c, 2, r]

    with tile.TileContext(nc) as tc:
        with (
            tc.tile_pool(name="consts", bufs=1) as consts,
            tc.tile_pool(name="persist", bufs=1) as persist,
            tc.tile_pool(name="work", bufs=5) as work,
            tc.tile_pool(name="ps_proj", bufs=2, space="PSUM") as ps_proj,
            tc.tile_pool(name="ps_s", bufs=3, space="PSUM") as ps_s,
            tc.tile_pool(name="ps_small", bufs=3, space="PSUM") as ps_small,
        ):
            # warm the ACT exp table before it's on the critical path
            dummy = consts.tile([P, 2], f32)
            nc.vector.memset(dummy, 0.0)
            nc.scalar.activation(
                out=dummy, in_=dummy, func=mybir.ActivationFunctionType.Exp
            )

            # x.T pieces interleaved with the pos tables they gate, in
            # consumption order, split across the two HWDGE queues
            HS = S // 2
            xT_sb = persist.tile([P, DCH, S], f32r)
            pq_sb = persist.tile([P, S], f32)
            pk_sb = persist.tile([P, S], f32)
            nc.sync.dma_start(out=xT_sb[:, :, 0:NSL], in_=xtr[0])
            nc.scalar.dma_start(out=xT_sb[:, :, NSL : 2 * NSL], in_=xtr[1])
            nc.sync.dma_start(out=pq_sb[:, 0:HS], in_=pq[:, 0:HS])
            nc.scalar.dma_start(out=pk_sb[:, 0:HS], in_=pk[:, 0:HS])
            nc.sync.dma_start(out=xT_sb[:, :, 2 * NSL : 3 * NSL], in_=xtr[2])
            nc.scalar.dma_start(out=xT_sb[:, :, 3 * NSL : S], in_=xtr[3])
            nc.sync.dma_start(out=pq_sb[:, HS:S], in_=pq[:, HS:S])
            nc.scalar.dma_start(out=pk_sb[:, HS:S], in_=pk[:, HS:S])

            # weights/consts on gpsimd's queue (small, land early)
            wq_sb = consts.tile([P, DCH, P], f32r)
            nc.gpsimd.dma_start(out=wq_sb, in_=wq[:])
            wk_sb = consts.tile([P, DCH, P], f32r)
            nc.gpsimd.dma_start(out=wk_sb, in_=wk[:])
            wv_sb = consts.tile([P, DCH, KD], f32r)
            nc.gpsimd.dma_start(out=wv_sb, in_=wv[:])
            bv_sb = consts.tile([KD, 1], f32)
            nc.gpsimd.dma_start(out=bv_sb, in_=bv[:])
            ident_sb = consts.tile([KD, KD], f32)
            nc.gpsimd.dma_start(out=ident_sb, in_=ident[:])
            msk_sb = consts.tile([P, 2, P], f32)
            nc.gpsimd.dma_start(out=msk_sb, in_=msk[:])

            # warm the PE (HAM clock gate) while the first DMA pieces land:
            # junk matmuls on a zeroed tile, never read back
            wdum = consts.tile([P, NSL], f32r)
            zeros0_sb = consts.tile([P, 1], f32)
            nc.vector.memset(zeros0_sb, 0.0)
            nc.vector.tensor_copy(wdum, zeros0_sb.to_broadcast((P, NSL)))
            ps_dum = ps_s.tile([P, 2 * P], f32, tag="s", name="ps_dum")
            for _ in range(44):
                nc.tensor.matmul(
                    ps_dum, wdum[:, 0:P], wdum[:, 0 : 2 * P],
                    start=True, stop=True,
                )

            # qT padded by one block so every sT matmul is a uniform N=256
            qT_sb = persist.tile([P, S + P], f32r)
            kT_sb = persist.tile([P, S], f32r)
            vT_sb = persist.tile([KD, S], f32)
            zeros_sb = consts.tile([P, 1], f32)
            nc.vector.memset(zeros_sb, 0.0)
            nc.vector.tensor_copy(
                qT_sb[:, S : S + P], zeros_sb.to_broadcast((P, P))
            )

            # v_aug[key, block, 0:64] = v; col 64 = 1.0 (rowsum); col 65 pad
            v_aug = persist.tile([P, NB, KD + 2], f32r)
            ones_sb = consts.tile([P, 1], f32)
            nc.vector.memset(ones_sb, 1.0)
            nc.vector.tensor_copy(
                v_aug[:, :, KD : KD + 2], ones_sb.to_broadcast((P, NB, 2))
            )

            # per-query-block normalized outputs, DMA'd out 4 blocks at a time
            out_all = persist.tile([P, NB, KD], f32)
            out_r = out.rearrange("(q r) k -> r q k", r=P)

            def proj_piece(n):
                sl = slice(n * NSL, (n + 1) * NSL)
                for grp in range(3):  # 0=q, 1=k, 2=v
                    w_g = (wq_sb, wk_sb, wv_sb)[grp]
                    m = P if grp < 2 else KD
                    ps = ps_proj.tile([m, NSL], f32, tag="ps", name="ps")
                    for c in range(DCH):
                        nc.tensor.matmul(
                            ps,
                            w_g[:, c, :m],
                            xT_sb[:, c, sl],
                            start=(c == 0),
                            stop=(c == DCH - 1),
                        )
                    if grp == 0:
                        nc.vector.tensor_add(qT_sb[:, sl], ps, pq_sb[:, sl])
                    elif grp == 1:
                        nc.vector.tensor_add(kT_sb[:, sl], ps, pk_sb[:, sl])
                    else:
                        nc.vector.tensor_scalar_add(vT_sb[:, sl], ps, bv_sb)

            def transpose_v(t):
                tp = ps_small.tile([P, KD], f32, tag="small", name="tp")
                nc.tensor.transpose(tp, vT_sb[:, ts(t, P)], ident_sb)
                nc.vector.tensor_copy(v_aug[:, t, 0:KD], tp)

            p_tiles = {}

            def score_block(kb):
                # sT_kb[c, r]: keys of block kb vs queries of blocks kb,kb+1
                s_ps = ps_s.tile([P, 2 * P], f32, tag="s", name="s_ps")
                nc.tensor.matmul(
                    s_ps,
                    kT_sb[:, ts(kb, P)],
                    qT_sb[:, kb * P : kb * P + 2 * P],
                    start=True, stop=True,
                )
                p_sb = work.tile([P, 2, P], f32r, tag="p_sb")
                nc.scalar.activation(
                    out=p_sb, in_=s_ps.rearrange("c (h r) -> c h r", h=2),
                    func=mybir.ActivationFunctionType.Exp,
                )
                # band+causal mask: half 0 keeps keys c <= r (diag block of
                # qb=kb), half 1 keeps c >= r (off-diag block of qb=kb+1)
                nc.vector.tensor_mul(p_sb, p_sb, msk_sb)
                p_tiles[kb] = p_sb

            def attend(qb):
                o_ps = ps_small.tile([P, KD + 2], f32, tag="small", name="o_ps")
                halves = [(p_tiles[qb], 0, qb)]
                if qb > 0:
                    halves.insert(0, (p_tiles[qb - 1], 1, qb - 1))
                for i, (pt, h, kb2) in enumerate(halves):
                    nc.tensor.matmul(
                        o_ps,
                        pt[:, h, :],
                        v_aug[:, kb2, :],
                        start=(i == 0),
                        stop=(i == len(halves) - 1),
                    )
                if qb > 1:
                    p_tiles.pop(qb - 2, None)
                r_sb = work.tile([P, 1], f32, tag="r_sb")
                nc.vector.reciprocal(r_sb, o_ps[:, KD : KD + 1])
                nc.scalar.activation(
                    out=out_all[:, qb, :], in_=o_ps[:, 0:KD],
                    func=mybir.ActivationFunctionType.Copy, scale=r_sb,
                )
                if qb % 4 == 3:
                    nc.sync.dma_start(
                        out=out_r[:, qb - 3 : qb + 1, :],
                        in_=out_all[:, qb - 3 : qb + 1, :],
                    )

            # ---- software-pipelined schedule over the 4 column pieces
            scored = 0
            attended = 0
            for n in range(NCH):
                proj_piece(n)
                for t in range(4 * n, 4 * (n + 1)):
                    transpose_v(t)
                target = min(4 * n + 2, NB - 1) if n < NCH - 1 else NB - 1
                while scored <= target:
                    score_block(scored)
                    scored += 1
                    if scored - attended > 2:
                        attend(attended)
                        attended += 1
            while attended < NB:
                attend(attended)
                attended += 1

    nc.finalize()
    return nc


def _prep_core_inputs(inputs):
    g = lambda k: np.asarray(inputs[k], dtype=np.float32)
    x = g("x")
    scale = 1.0 / np.sqrt(np.float32(KD))
    temp = float(np.asarray(inputs["temperature"]).reshape(-1)[0])
    alpha = scale * temp  # folded (softmax temp) * (score scale)

    wq = np.concatenate([g("Wqr"), g("Wqi")], axis=1) * (scale * alpha)
    pq = np.concatenate(
        [
            g("pos_qr") * alpha + g("bqr") * (scale * alpha),
            g("pos_qi") * alpha + g("bqi") * (scale * alpha),
        ],
        axis=1,
    ).T  # [128, S]
    wk = np.concatenate([g("Wkr"), -g("Wki")], axis=1)
    pk = np.concatenate(
        [g("pos_kr") + g("bkr"), -(g("pos_ki") + g("bki"))], axis=1
    ).T
    wv = g("Wv")
    bv = g("bv").reshape(KD, 1)

    pe_pack = lambda w: np.ascontiguousarray(
        w.reshape(DCH, P, w.shape[1]).transpose(1, 0, 2), dtype=np.float32
    )
    shared = {
        "wq": pe_pack(wq),
        "wk": pe_pack(wk),
        "wv": pe_pack(wv),
        "pq": np.ascontiguousarray(pq, dtype=np.float32),
        "pk": np.ascontiguousarray(pk, dtype=np.float32),
        "bv": np.ascontiguousarray(bv, dtype=np.float32),
    }
    in_maps = []
    for b in range(NCORES):
        m = dict(shared)
        # xtr[n, p, c, j] = x[b].T[c*128+p, n*512+j]
        xT_b = np.ascontiguousarray(x[b].T, dtype=np.float32)
        m["xtr"] = np.ascontiguousarray(
            xT_b.reshape(DCH, P, NCH, NSL).transpose(2, 1, 0, 3)
        )
        in_maps.append(m)
    return in_maps


def kernel(**inputs):
    from concourse.bass_utils import run_bass_kernel_spmd

    nc = _CACHE.get("nc")
    if nc is None:
        nc = _CACHE["nc"] = _build_nc()
    in_maps = _prep_core_inputs(inputs)
    res = run_bass_kernel_spmd(
        nc, in_maps, core_ids=list(range(NCORES)), **TRACE_KWARGS
    )
    _CACHE["last_result"] = res
    return np.stack([res.results[b]["out"] for b in range(NCORES)], axis=0)
